# revision 1
# baseline (speedup 1.0000x reference)
"""Trainium2 Bass kernel for nn_DiagMean (histogram_binning).

Computes, per batch image A [T, T]: the mean over each diagonal d = j - i
(for |d| <= T/2, excluding the last element of each diagonal), then centers
across the T+1 diagonal bins and negates.

Strategy (pure data-parallel over batch, 2 images per core on 8 cores):
  - Skewed DMA: for a 128-row tile starting at row i0, the access pattern
    [partition stride T+1, free stride 1] reads S[p, k] = A[i0+p, i0+p+k-T/2],
    so diagonal bin k is a *column* of S. Each partition's read is contiguous
    in HBM, so DMA runs at full bandwidth.
  - Columns fully outside any valid diagonal are never loaded (per-tile
    [kmin, kmax] window): ~22% HBM traffic saved.
  - The trapezoid validity boundary reduces to two shared 128x128 masks
    (prefix: p+m >= 128, suffix: p+m <= 126) applied to one 128-wide column
    strip per side per tile.
  - Column sums via TensorE: ones[128,1].T @ S chunk -> PSUM [1, N],
    accumulated over the 16 row-tiles of an image (fp32r, 1 cycle/col).
  - Tiny epilogue on VectorE: negmean = psum * (-1/count), centering, DMA out.
"""

import numpy as np

import concourse.bacc as bacc
import concourse.bass as bass
import concourse.mybir as mybir
import concourse.tile as tile
from concourse.bass_utils import run_bass_kernel_spmd

B, T = 16, 2048
HALF = T // 2          # 1024
K = T + 1              # 2049 diagonal bins
N_CORES = 8
BPC = B // N_CORES     # images per core
P = 128
NT = T // P            # row tiles per image
BANK = 512             # fp32 elements per PSUM bank

_nc_cache = None


def _build():
    nc = bacc.Bacc("TRN2", target_bir_lowering=False, debug=False)
    f32 = mybir.dt.float32
    f32r = mybir.dt.float32r

    # Front-padded with HALF zeros so tile 0's skewed read of row 0 lands in
    # the pad instead of out of bounds (the pad zeros are exactly the masked
    # region, so no special-casing).
    x = nc.dram_tensor("x", [HALF + BPC * T * T], f32, kind="ExternalInput")
    y = nc.dram_tensor("y", [BPC, K], f32, kind="ExternalOutput")

    pp = np.arange(P)[:, None]
    mm = np.arange(P)[None, :]
    maskP_np = (pp + mm >= P).astype(np.float32)        # prefix validity
    maskS_np = (pp + mm <= P - 2).astype(np.float32)    # suffix validity
    counts = (T - 1 - np.abs(np.arange(-HALF, HALF + 1))).astype(np.float64)
    # Pad row length to even so SBUF partition strides stay 8-byte aligned
    # (TENSOR_TENSOR_REDUCE rejects odd f32 strides).
    KP = K + 1
    negrecip_np = np.zeros((1, KP), dtype=np.float32)
    negrecip_np[0, :K] = (-1.0 / counts).astype(np.float32)
    ones_np = np.ones((P, 1), dtype=np.float32)
    zrhs_np = np.zeros((2, BANK), dtype=np.float32)

    maskP_d = nc.inline_tensor(maskP_np, name="maskP")
    maskS_d = nc.inline_tensor(maskS_np, name="maskS")
    negrecip_d = nc.inline_tensor(negrecip_np, name="negrecip")
    ones_d = nc.inline_tensor(ones_np, name="onesw")
    zrhs_d = nc.inline_tensor(zrhs_np, name="zrhs")

    # Per-tile valid column windows and PSUM-bank chunk first/last writers.
    tinfo = []
    for t in range(NT):
        i0 = t * P
        kmin = max(0, 896 - i0)
        kmax = min(T, 3070 - i0)
        tinfo.append((i0, kmin, kmax))
    NCH = T // BANK                                     # 4 (columns 0..2047)
    first = {}
    last = {}
    for t, (i0, kmin, kmax) in enumerate(tinfo):
        for c in range(NCH):
            lo, hi = c * BANK, c * BANK + BANK - 1
            if kmin <= hi and min(kmax, T - 1) >= lo:
                first.setdefault(c, t)
                last[c] = t
        if kmax == T:  # column k=2048 (diag +1024) -> separate 2-wide psum
            first.setdefault("c4", t)
            last["c4"] = t

    with tile.TileContext(nc) as tc:
        with (
            tc.tile_pool(name="consts", bufs=1) as consts,
            tc.tile_pool(name="data", bufs=12) as data,
            tc.tile_pool(name="acc", bufs=1, space=bass.MemorySpace.PSUM) as accp,
            tc.tile_pool(name="post", bufs=1) as post,
        ):
            maskP = consts.tile([P, P], f32r)
            maskS = consts.tile([P, P], f32r)
            negrecip = consts.tile([1, KP], f32)
            ones = consts.tile([P, 1], f32r)
            zrhs = consts.tile([2, BANK], f32r)
            nc.gpsimd.dma_start(maskP[:], maskP_d[:].bitcast(f32r))
            nc.gpsimd.dma_start(maskS[:], maskS_d[:].bitcast(f32r))
            nc.gpsimd.dma_start(negrecip[:], negrecip_d[:])
            nc.gpsimd.dma_start(ones[:], ones_d[:].bitcast(f32r))
            nc.gpsimd.dma_start(zrhs[:], zrhs_d[:].bitcast(f32r))
            # Collector: touch each const on VectorE once so later DVE ops
            # inherit the const-DMA sync via engine program order instead of
            # each carrying its own semaphore wait (ISA wait-slot limit).
            warm = consts.tile([P, P], f32)
            nc.vector.tensor_copy(warm[:], maskP[:].bitcast(f32))
            nc.vector.tensor_copy(warm[:], maskS[:].bitcast(f32))
            warm2 = consts.tile([1, KP], f32)
            nc.vector.tensor_copy(warm2[:], negrecip[:])

            # PSUM budget is 8 banks of 512 f32: image 0's 2048 main
            # columns in banks 0-3, image 1's chunks 1-3 in banks 4-6, both
            # images' k=2048 accumulators share bank 7, and image 1's chunk 0
            # (first written at its tile 4) reuses bank 0 after image 0's
            # epilogue has read it (WAR handled by the tile framework).
            psumA = accp.tile([1, T], f32, tag="psumA")
            psumB = accp.tile([1, 3 * BANK], f32, tag="psumB")
            psum2 = accp.tile([1, BANK], f32, tag="psum2")
            # TENSOR_TENSOR_REDUCE requires partition-base-0 operands, so
            # each image gets its own partition-0 scratch tiles.
            nm_t = [post.tile([1, KP], f32, tag=f"nm{i}", name=f"nm{i}") for i in range(BPC)]
            s1_t = [post.tile([1, 1], f32, tag=f"s1{i}", name=f"s1{i}") for i in range(BPC)]
            av_t = [post.tile([1, 1], f32, tag=f"av{i}", name=f"av{i}") for i in range(BPC)]
            ot_t = [post.tile([1, KP], f32, tag=f"ot{i}", name=f"ot{i}") for i in range(BPC)]
            pp_t = [post.tile([1, 4], f32, tag=f"pp{i}", name=f"pp{i}") for i in range(BPC)]
            psumA2 = None
            for b in range(BPC):
                base = b * T * T
                nm, s1, avgneg, ot = nm_t[b], s1_t[b], av_t[b], ot_t[b]
                if b == 1:
                    psumA2 = accp.tile([1, BANK], f32, tag="psumA")

                def chunk_dst(c, lo, hi, b=b):
                    # psum AP for chunk c columns [lo, hi] of image b
                    if b == 0:
                        return psumA[0:1, lo : hi + 1]
                    if c == 0:
                        return psumA2[0:1, lo : hi + 1]
                    return psumB[0:1, lo - BANK : hi + 1 - BANK]

                # A start=True matmul arms zeroing for its whole 2KB PSUM
                # bank, so every accumulation group begins with one full-bank
                # zeroing matmul; all data matmuls then accumulate.
                # Image 0's k=2048 group borrows psumB's first bank (idle
                # until image 1's chunks start); image 1 uses psum2's bank.
                c4dst = psumB if b == 0 else psum2
                for dst in [chunk_dst(c, c * BANK, c * BANK + BANK - 1) for c in range(NCH)] + [
                    c4dst[0:1, 0:BANK]
                ]:
                    nc.tensor.matmul(
                        dst,
                        ones[0:2, :],
                        zrhs[:],
                        start=True,
                        stop=False,
                    )
                for t, (i0, kmin, kmax) in enumerate(tinfo):
                    rows = P - 1 if t == NT - 1 else P
                    # fp32r matmul needs even N; suffix-capped kmax is even, so
                    # load one extra column there (maskS's last column zeroes it).
                    kmax_dma = kmax + 1 if i0 >= 1024 else kmax
                    width = kmax_dma - kmin + 1
                    S = data.tile([P, KP], f32r, tag="S")
                    dma_eng = nc.sync if (b * NT + t) % 2 == 0 else nc.scalar
                    dma_eng.dma_start(
                        S[0:rows, kmin : kmax_dma + 1],
                        bass.AP(
                            x,
                            base + i0 * (T + 1) + kmin,
                            [[T + 1, rows], [1, width]],
                        ).bitcast(f32r),
                    )
                    if i0 <= 896:
                        a = 896 - i0
                        nc.vector.tensor_mul(
                            S[0:rows, a : a + P],
                            S[0:rows, a : a + P],
                            maskP[0:rows, :],
                        )
                    if i0 >= 896:
                        a = 2944 - i0
                        w = kmax_dma + 1 - a
                        nc.vector.tensor_mul(
                            S[0:rows, a : a + w],
                            S[0:rows, a : a + w],
                            maskS[0:rows, 0:w],
                        )
                    for c in range(NCH):
                        lo = max(kmin, c * BANK)
                        hi = min(min(kmax, T - 1), c * BANK + BANK - 1)
                        if lo > hi:
                            continue
                        if (hi - lo + 1) % 2 == 1:
                            hi += 1
                            assert hi <= min(kmax_dma, c * BANK + BANK - 1)
                        nc.tensor.matmul(
                            chunk_dst(c, lo, hi),
                            ones[0:rows, :],
                            S[0:rows, lo : hi + 1],
                            start=False,
                            stop=(last[c] == t),
                        )
                    if kmax == T:
                        # fp32r matmul needs N>=2: recompute col 2047 into a
                        # scratch lane and keep only col 2048's sum.
                        nc.tensor.matmul(
                            c4dst[0:1, 0:2],
                            ones[0:rows, :],
                            S[0:rows, T - 1 : T + 1],
                            start=False,
                            stop=(last["c4"] == t),
                        )
                # Per-image epilogue, chunked so most of it hides under
                # remaining DMAs: col 2048 closes at tile 7, columns
                # [1536,2048) at tile 11, the rest only at tile 15. Chain the
                # fused multiply+reduce passes through s1.
                nc.vector.tensor_mul(
                    nm[0:1, T : T + 1],
                    c4dst[0:1, 1:2],
                    negrecip[:, T : T + 1],
                )
                # negmean per chunk (chunk 3 hides under DMAs; 0 and 1
                # close only at the last tile). PSUM-sourced TT is DVE-only.
                # ACT sums each chunk (Identity + accum_out) as soon as its
                # TT lands, so only the last chunk's sum is on the tail.
                scr, pp = ot, pp_t[b]  # ot doubles as ACT dump pre-centering
                nc.vector.tensor_mul(
                    nm[0:1, 3 * BANK : T],
                    chunk_dst(3, 3 * BANK, T - 1),
                    negrecip[:, 3 * BANK : T],
                )
                nc.scalar.activation(
                    scr[0:1, 3 * BANK : T],
                    nm[0:1, 3 * BANK : T],
                    mybir.ActivationFunctionType.Identity,
                    accum_out=pp[0:1, 3:4],
                )
                nc.vector.tensor_mul(
                    nm[0:1, BANK : 3 * BANK],
                    chunk_dst(1, BANK, 3 * BANK - 1),
                    negrecip[:, BANK : 3 * BANK],
                )
                nc.scalar.activation(
                    scr[0:1, BANK : 3 * BANK],
                    nm[0:1, BANK : 3 * BANK],
                    mybir.ActivationFunctionType.Identity,
                    accum_out=pp[0:1, 1:2],
                )
                nc.vector.tensor_mul(
                    nm[0:1, 0:BANK],
                    chunk_dst(0, 0, BANK - 1),
                    negrecip[:, 0:BANK],
                )
                nc.scalar.activation(
                    scr[0:1, 0:BANK],
                    nm[0:1, 0:BANK],
                    mybir.ActivationFunctionType.Identity,
                    accum_out=pp[0:1, 0:1],
                )
                nc.vector.tensor_copy(pp[0:1, 2:3], nm[0:1, T : T + 1])
                nc.vector.reduce_sum(
                    s1[0:1, 0:1], pp[0:1, 0:4], axis=mybir.AxisListType.X
                )
                nc.vector.tensor_scalar_mul(
                    avgneg[0:1, 0:1], s1[0:1, 0:1], -1.0 / K
                )
                # Centering: image 0's full-width pass hides under image 1's
                # DMAs; image 1's is split across DVE and ACT (independent
                # SBUF ports) to halve the tail.
                HK = 1025
                if b == 0:
                    nc.vector.tensor_scalar_add(
                        ot[0:1, 0:K], nm[0:1, 0:K], avgneg[0:1, 0:1]
                    )
                else:
                    nc.vector.tensor_scalar_add(
                        ot[0:1, 0:HK],
                        nm[0:1, 0:HK],
                        avgneg[0:1, 0:1],
                    )
                    nc.scalar.activation(
                        ot[0:1, HK:K],
                        nm[0:1, HK:K],
                        mybir.ActivationFunctionType.Identity,
                        bias=avgneg[0:1, 0:1],
                        scale=1.0,
                    )
                if b == 0:
                    nc.sync.dma_start(y[b : b + 1, :], ot[0:1, 0:K])
                else:
                    nc.sync.dma_start(y[b : b + 1, 0:HK], ot[0:1, 0:HK])
                    nc.scalar.dma_start(y[b : b + 1, HK:K], ot[0:1, HK:K])
    nc.compile()
    return nc


def kernel(**inputs: np.ndarray) -> np.ndarray:
    global _nc_cache
    x = np.asarray(inputs["inputs"], dtype=np.float32)
    assert x.shape == (B, T, T)
    if _nc_cache is None:
        _nc_cache = _build()
    pad = np.zeros(HALF, dtype=np.float32)
    in_maps = [
        {
            "x": np.concatenate(
                [pad, np.ascontiguousarray(x[c * BPC : (c + 1) * BPC]).reshape(-1)]
            )
        }
        for c in range(N_CORES)
    ]
    res = run_bass_kernel_spmd(_nc_cache, in_maps, core_ids=list(range(N_CORES)))
    return np.concatenate([r["y"] for r in res.results], axis=0)



# revision 6
# speedup vs baseline: 29.2831x; 29.2831x over previous
"""Trainium2 Bass kernel for nn_DiagMean (histogram_binning).

Computes, per batch image A [T, T]: the mean over each diagonal d = j - i
(for |d| <= T/2, excluding the last element of each diagonal), then centers
across the T+1 diagonal bins and negates.

Strategy (pure data-parallel over batch, 2 images per core on 8 cores):
  - Skewed DMA: for a 128-row tile starting at row i0, the access pattern
    [partition stride T+1, free stride 1] reads S[p, k] = A[i0+p, i0+p+k-T/2],
    so diagonal bin k is a *column* of S. Each partition's read is contiguous
    in HBM, so DMA runs at full bandwidth.
  - Columns fully outside any valid diagonal are never loaded (per-tile
    [kmin, kmax] window): ~22% HBM traffic saved.
  - The trapezoid validity boundary reduces to two shared 128x128 masks
    (prefix: p+m >= 128, suffix: p+m <= 126) applied to one 128-wide column
    strip per side per tile.
  - Column sums via TensorE: ones[128,1].T @ S chunk -> PSUM [1, N],
    accumulated over the 16 row-tiles of an image (fp32r, 1 cycle/col).
  - Tiny epilogue on VectorE: negmean = psum * (-1/count), centering, DMA out.
"""

import numpy as np

import concourse.bacc as bacc
import concourse.bass as bass
import concourse.mybir as mybir
import concourse.tile as tile
from concourse.bass_utils import run_bass_kernel_spmd

B, T = 16, 2048
HALF = T // 2          # 1024
K = T + 1              # 2049 diagonal bins
N_CORES = 8
BPC = B // N_CORES     # images per core
P = 128
NT = T // P            # row tiles per image
BANK = 512             # fp32 elements per PSUM bank

_nc_cache = None


def _build(repeat=1):
    # repeat>1 re-runs the whole compute loop inside one NEFF (bench-only:
    # lets a slope fit separate HW time from dispatch overhead).
    nc = bacc.Bacc("TRN2", target_bir_lowering=False, debug=False)
    f32 = mybir.dt.float32
    f32r = mybir.dt.float32r

    # Front-padded with HALF zeros so tile 0's skewed read of row 0 lands in
    # the pad instead of out of bounds (the pad zeros are exactly the masked
    # region, so no special-casing).
    x = nc.dram_tensor("x", [HALF + BPC * T * T], f32, kind="ExternalInput")
    y = nc.dram_tensor("y", [BPC, K], f32, kind="ExternalOutput")

    pp = np.arange(P)[:, None]
    mm = np.arange(P)[None, :]
    maskP_np = (pp + mm >= P).astype(np.float32)        # prefix validity
    maskS_np = (pp + mm <= P - 2).astype(np.float32)    # suffix validity
    counts = (T - 1 - np.abs(np.arange(-HALF, HALF + 1))).astype(np.float64)
    # Pad row length to even so SBUF partition strides stay 8-byte aligned
    # (TENSOR_TENSOR_REDUCE rejects odd f32 strides).
    KP = K + 1
    negrecip_np = np.zeros((1, KP), dtype=np.float32)
    negrecip_np[0, :K] = (-1.0 / counts).astype(np.float32)
    ones_np = np.ones((P, 1), dtype=np.float32)
    zrhs_np = np.zeros((2, BANK), dtype=np.float32)

    maskP_d = nc.inline_tensor(maskP_np, name="maskP")
    maskS_d = nc.inline_tensor(maskS_np, name="maskS")
    negrecip_d = nc.inline_tensor(negrecip_np, name="negrecip")
    ones_d = nc.inline_tensor(ones_np, name="onesw")
    zrhs_d = nc.inline_tensor(zrhs_np, name="zrhs")

    # Per-tile valid column windows and PSUM-bank chunk first/last writers.
    tinfo = []
    for t in range(NT):
        i0 = t * P
        kmin = max(0, 896 - i0)
        kmax = min(T, 3070 - i0)
        tinfo.append((i0, kmin, kmax))
    NCH = T // BANK                                     # 4 (columns 0..2047)
    first = {}
    last = {}
    for t, (i0, kmin, kmax) in enumerate(tinfo):
        for c in range(NCH):
            lo, hi = c * BANK, c * BANK + BANK - 1
            if kmin <= hi and min(kmax, T - 1) >= lo:
                first.setdefault(c, t)
                last[c] = t
        if kmax == T:  # column k=2048 (diag +1024) -> separate 2-wide psum
            first.setdefault("c4", t)
            last["c4"] = t

    with tile.TileContext(nc) as tc:
        with (
            tc.tile_pool(name="consts", bufs=1) as consts,
            tc.tile_pool(name="data", bufs=12) as data,
            tc.tile_pool(name="acc", bufs=1, space=bass.MemorySpace.PSUM) as accp,
            tc.tile_pool(name="post", bufs=1) as post,
        ):
            maskP = consts.tile([P, P], f32r)
            maskS = consts.tile([P, P], f32r)
            negrecip = consts.tile([1, KP], f32)
            ones = consts.tile([P, 1], f32r)
            zrhs = consts.tile([2, BANK], f32r)
            nc.gpsimd.dma_start(maskP[:], maskP_d[:].bitcast(f32r))
            nc.gpsimd.dma_start(maskS[:], maskS_d[:].bitcast(f32r))
            nc.gpsimd.dma_start(negrecip[:], negrecip_d[:])
            nc.gpsimd.dma_start(ones[:], ones_d[:].bitcast(f32r))
            nc.gpsimd.dma_start(zrhs[:], zrhs_d[:].bitcast(f32r))
            # Collector: touch each const on VectorE once so later DVE ops
            # inherit the const-DMA sync via engine program order instead of
            # each carrying its own semaphore wait (ISA wait-slot limit).
            warm = consts.tile([P, P], f32)
            nc.vector.tensor_copy(warm[:], maskP[:].bitcast(f32))
            nc.vector.tensor_copy(warm[:], maskS[:].bitcast(f32))
            warm2 = consts.tile([1, KP], f32)
            nc.vector.tensor_copy(warm2[:], negrecip[:])

            # PSUM budget is 8 banks of 512 f32: image 0's 2048 main
            # columns in banks 0-3, image 1's chunks 1-3 in banks 4-6, both
            # images' k=2048 accumulators share bank 7, and image 1's chunk 0
            # (first written at its tile 4) reuses bank 0 after image 0's
            # epilogue has read it (WAR handled by the tile framework).
            # TENSOR_TENSOR_REDUCE requires partition-base-0 operands, so
            # each image gets its own partition-0 scratch tiles.
            nm_t = [post.tile([1, KP], f32, tag=f"nm{i}", name=f"nm{i}") for i in range(BPC)]
            s1_t = [post.tile([1, 1], f32, tag=f"s1{i}", name=f"s1{i}") for i in range(BPC)]
            av_t = [post.tile([1, 1], f32, tag=f"av{i}", name=f"av{i}") for i in range(BPC)]
            ot_t = [post.tile([1, KP], f32, tag=f"ot{i}", name=f"ot{i}") for i in range(BPC)]
            pp_t = [post.tile([1, 4], f32, tag=f"pp{i}", name=f"pp{i}") for i in range(BPC)]
            psumA2 = None
            for rep in range(repeat):
              psumA = accp.tile([1, T], f32, tag="psumA")
              psumB = accp.tile([1, 3 * BANK], f32, tag="psumB")
              psum2 = accp.tile([1, BANK], f32, tag="psum2")
              for b in range(BPC):
                base = b * T * T
                nm, s1, avgneg, ot = nm_t[b], s1_t[b], av_t[b], ot_t[b]
                if b == 1:
                    psumA2 = accp.tile([1, BANK], f32, tag="psumA")

                def chunk_dst(c, lo, hi, b=b):
                    # psum AP for chunk c columns [lo, hi] of image b
                    if b == 0:
                        return psumA[0:1, lo : hi + 1]
                    if c == 0:
                        return psumA2[0:1, lo : hi + 1]
                    return psumB[0:1, lo - BANK : hi + 1 - BANK]

                # A start=True matmul arms zeroing for its whole 2KB PSUM
                # bank, so every accumulation group begins with one full-bank
                # zeroing matmul; all data matmuls then accumulate.
                # Image 0's k=2048 group borrows psumB's first bank (idle
                # until image 1's chunks start); image 1 uses psum2's bank.
                c4dst = psumB if b == 0 else psum2
                for dst in [chunk_dst(c, c * BANK, c * BANK + BANK - 1) for c in range(NCH)] + [
                    c4dst[0:1, 0:BANK]
                ]:
                    nc.tensor.matmul(
                        dst,
                        ones[0:2, :],
                        zrhs[:],
                        start=True,
                        stop=False,
                    )
                for t, (i0, kmin, kmax) in enumerate(tinfo):
                    rows = P - 1 if t == NT - 1 else P
                    # fp32r matmul needs even N; suffix-capped kmax is even, so
                    # load one extra column there (maskS's last column zeroes it).
                    kmax_dma = kmax + 1 if i0 >= 1024 else kmax
                    width = kmax_dma - kmin + 1
                    S = data.tile([P, KP], f32r, tag="S")
                    dma_eng = nc.sync if (b * NT + t) % 2 == 0 else nc.scalar
                    dma_eng.dma_start(
                        S[0:rows, kmin : kmax_dma + 1],
                        bass.AP(
                            x,
                            base + i0 * (T + 1) + kmin,
                            [[T + 1, rows], [1, width]],
                        ).bitcast(f32r),
                    )
                    if i0 <= 896:
                        a = 896 - i0
                        nc.vector.tensor_mul(
                            S[0:rows, a : a + P],
                            S[0:rows, a : a + P],
                            maskP[0:rows, :],
                        )
                    if i0 >= 896:
                        a = 2944 - i0
                        w = kmax_dma + 1 - a
                        nc.vector.tensor_mul(
                            S[0:rows, a : a + w],
                            S[0:rows, a : a + w],
                            maskS[0:rows, 0:w],
                        )
                    for c in range(NCH):
                        lo = max(kmin, c * BANK)
                        hi = min(min(kmax, T - 1), c * BANK + BANK - 1)
                        if lo > hi:
                            continue
                        if (hi - lo + 1) % 2 == 1:
                            hi += 1
                            assert hi <= min(kmax_dma, c * BANK + BANK - 1)
                        nc.tensor.matmul(
                            chunk_dst(c, lo, hi),
                            ones[0:rows, :],
                            S[0:rows, lo : hi + 1],
                            start=False,
                            stop=(last[c] == t),
                        )
                    if kmax == T:
                        # fp32r matmul needs N>=2: recompute col 2047 into a
                        # scratch lane and keep only col 2048's sum.
                        nc.tensor.matmul(
                            c4dst[0:1, 0:2],
                            ones[0:rows, :],
                            S[0:rows, T - 1 : T + 1],
                            start=False,
                            stop=(last["c4"] == t),
                        )
                # Per-image epilogue, chunked so most of it hides under
                # remaining DMAs: col 2048 closes at tile 7, columns
                # [1536,2048) at tile 11, the rest only at tile 15. Chain the
                # fused multiply+reduce passes through s1.
                nc.vector.tensor_mul(
                    nm[0:1, T : T + 1],
                    c4dst[0:1, 1:2],
                    negrecip[:, T : T + 1],
                )
                # negmean per chunk (chunk 3 hides under DMAs; 0 and 1
                # close only at the last tile). PSUM-sourced TT is DVE-only.
                # ACT sums each chunk (Identity + accum_out) as soon as its
                # TT lands, so only the last chunk's sum is on the tail.
                scr, pp = ot, pp_t[b]  # ot doubles as ACT dump pre-centering
                nc.vector.tensor_mul(
                    nm[0:1, 3 * BANK : T],
                    chunk_dst(3, 3 * BANK, T - 1),
                    negrecip[:, 3 * BANK : T],
                )
                nc.scalar.activation(
                    scr[0:1, 3 * BANK : T],
                    nm[0:1, 3 * BANK : T],
                    mybir.ActivationFunctionType.Identity,
                    accum_out=pp[0:1, 3:4],
                )
                nc.vector.tensor_mul(
                    nm[0:1, BANK : 3 * BANK],
                    chunk_dst(1, BANK, 3 * BANK - 1),
                    negrecip[:, BANK : 3 * BANK],
                )
                nc.scalar.activation(
                    scr[0:1, BANK : 3 * BANK],
                    nm[0:1, BANK : 3 * BANK],
                    mybir.ActivationFunctionType.Identity,
                    accum_out=pp[0:1, 1:2],
                )
                nc.vector.tensor_mul(
                    nm[0:1, 0:BANK],
                    chunk_dst(0, 0, BANK - 1),
                    negrecip[:, 0:BANK],
                )
                nc.scalar.activation(
                    scr[0:1, 0:BANK],
                    nm[0:1, 0:BANK],
                    mybir.ActivationFunctionType.Identity,
                    accum_out=pp[0:1, 0:1],
                )
                nc.vector.tensor_copy(pp[0:1, 2:3], nm[0:1, T : T + 1])
                nc.vector.reduce_sum(
                    s1[0:1, 0:1], pp[0:1, 0:4], axis=mybir.AxisListType.X
                )
                nc.vector.tensor_scalar_mul(
                    avgneg[0:1, 0:1], s1[0:1, 0:1], -1.0 / K
                )
                # Centering: image 0's full-width pass hides under image 1's
                # DMAs; image 1's is split across DVE and ACT (independent
                # SBUF ports) to halve the tail.
                HK = 1025
                if b == 0:
                    nc.vector.tensor_scalar_add(
                        ot[0:1, 0:K], nm[0:1, 0:K], avgneg[0:1, 0:1]
                    )
                else:
                    nc.vector.tensor_scalar_add(
                        ot[0:1, 0:HK],
                        nm[0:1, 0:HK],
                        avgneg[0:1, 0:1],
                    )
                    nc.scalar.activation(
                        ot[0:1, HK:K],
                        nm[0:1, HK:K],
                        mybir.ActivationFunctionType.Identity,
                        bias=avgneg[0:1, 0:1],
                        scale=1.0,
                    )
                if b == 0:
                    nc.sync.dma_start(y[b : b + 1, :], ot[0:1, 0:K])
                else:
                    nc.sync.dma_start(y[b : b + 1, 0:HK], ot[0:1, 0:HK])
                    nc.scalar.dma_start(y[b : b + 1, HK:K], ot[0:1, HK:K])
    nc.compile()
    return nc


def kernel(**inputs: np.ndarray) -> np.ndarray:
    global _nc_cache
    x = np.asarray(inputs["inputs"], dtype=np.float32)
    assert x.shape == (B, T, T)
    if _nc_cache is None:
        _nc_cache = _build()
    pad = np.zeros(HALF, dtype=np.float32)
    in_maps = [
        {
            "x": np.concatenate(
                [pad, np.ascontiguousarray(x[c * BPC : (c + 1) * BPC]).reshape(-1)]
            )
        }
        for c in range(N_CORES)
    ]
    res = run_bass_kernel_spmd(_nc_cache, in_maps, core_ids=list(range(N_CORES)))
    return np.concatenate([r["y"] for r in res.results], axis=0)

